# revision 27
# baseline (speedup 1.0000x reference)
"""Trainium2 Bass kernel for nn_MultiScaleFusionLayer (moe_routing).

Mathematical structure of the reference:
  - Every stage (expert matmuls, gate logits, mixture combine, attention
    softmax) is position-wise along L.
  - The final output is `task` (computed from gates at l=0 and attention
    scores at l=0..3) broadcast along L=100.
  => Only input positions l in {0,1,2,3} ever influence the output.

Strategy:
  - Host slices inputs to l<4 and shards batch B=2048 across 8 cores
    (256 rows/core, 4 positions => N=1024 "rows" per core, l-major).
  - Activations are shipped feature-on-partition (FT layout) with a ones
    row appended, so every bias folds into the matmuls and matmul lhsT
    (stationary operand) never needs on-device transposition.
  - One fused weight matrix computes, per 128-row tile: shared experts E,
    all 16 gate logits, and the id/img/txt-feature parts of all task
    experts in two K-accumulated matmuls per PSUM bank. The gate_share
    contribution to the task experts is added with one PE transpose + two
    accumulating matmuls.
  - Expert mixture (per-row gating): DVE multiplies by broadcast gate
    columns; the 4-expert reduction runs as pairwise adds on the
    otherwise-idle GpSimd engine. Attention runs as 2 PE transposes +
    matmul (bias via a ones-row K=1 matmul) + softmax (no max
    subtraction: logits are tiny) fused into an incremental task
    accumulation, so nothing but the output DMA remains after the loop.
  - Device returns task [256, 64] per core; host broadcasts to L=100.

Layouts: expert blocks are d-major (col = d*4 + e) so the e-reduction is
an innermost-axis reduce; `combined`/attention use g-major blocks
(col = g*64 + d) so every reduce/copy output is contiguous.

mm_dtype modes: "f32" (exact), "f32r" (fp32 data, 1 cycle/row matmuls,
requires on-device rounding copies), "bf16" (host-cast inputs, fastest).
"""

import sys

if "/opt/trn_rl_repo" not in sys.path:
    sys.path.insert(0, "/opt/trn_rl_repo")

import numpy as np

B, L, D = 2048, 100, 64
OUT_L = 100
NCORES = 8
BC = B // NCORES          # 256 batch rows per core
LK = 4                    # positions that matter
N = BC * LK               # 1024 rows per core (n = l*BC + b)
NT = N // 128             # 8 row-tiles of 128 rows
KA, KB = 128, 65          # xin partition split (192 features + ones)
WC = 1040                 # fused cols: E 256 | sels 16 | T1f 256 | T2f 256 | T3f 256
WCAT = 2000               # wcat_a cols: wbig 1040 | watt 128 | batt 64 | wgs2 768

_f32 = np.float32


def _np_dtype(mm_dtype):
    if mm_dtype == "bf16":
        import ml_dtypes
        return np.dtype(ml_dtypes.bfloat16)
    return np.dtype(np.float32)


def pack_weights(mm_dtype, Ws, bs, Wt1, bt1, Wt2, bt2, Wt3, bt3,
                 Wgs, bgs, Wg1, bg1, Wg2, bg2, Wg3, bg3, Watt, batt):
    """Build the fused device weight arrays (replicated per core)."""
    def blk(Wt):  # [E,Din,D] -> [Din, D*4] with col = d*4+e
        return np.ascontiguousarray(
            np.transpose(Wt, (1, 2, 0)).reshape(Wt.shape[1], -1))

    def bblk(bt):  # [E,D] -> [D*4]
        return np.ascontiguousarray(bt.T.reshape(-1))

    Wbig = np.zeros((193, WC), dtype=_f32)
    # cols: [E 0:256][sels 256:272][T1f 272:528][T2f 528:784][T3f 784:1040]
    Wbig[0:192, 0:256] = blk(Ws)
    Wbig[192, 0:256] = bblk(bs)
    Wbig[0:192, 256:260] = Wgs
    Wbig[192, 256:260] = bgs
    Wbig[0:64, 260:264] = Wg1
    Wbig[192, 260:264] = bg1
    Wbig[64:128, 264:268] = Wg2
    Wbig[192, 264:268] = bg2
    Wbig[128:192, 268:272] = Wg3
    Wbig[192, 268:272] = bg3
    Wbig[0:64, 272:528] = blk(Wt1)
    Wbig[192, 272:528] = bblk(bt1)
    Wbig[64:128, 528:784] = blk(Wt2)
    Wbig[192, 528:784] = bblk(bt2)
    Wbig[128:192, 784:1040] = blk(Wt3)
    Wbig[192, 784:1040] = bblk(bt3)

    wgs2 = np.concatenate([blk(Wt1), blk(Wt2), blk(Wt3)], axis=1)  # [64, 768]
    watt2 = np.concatenate([Watt[0:128], Watt[128:256]], axis=1)   # [128, 128]

    wcat_a = np.zeros((KA, WCAT), dtype=_f32)
    wcat_a[:, 0:1040] = Wbig[0:128]
    wcat_a[:, 1040:1168] = watt2
    wcat_a[64, 1168:1232] = batt  # row 64: matches xin_b's ones-row partition
    wcat_a[0:64, 1232:2000] = wgs2
    wcat_b = np.ascontiguousarray(Wbig[128:193])

    ddt = _np_dtype(mm_dtype)
    return {
        "wcat_a": wcat_a.astype(ddt),
        "wcat_b": wcat_b.astype(ddt),
        "ident": np.eye(128, dtype=_f32).astype(ddt),
    }


def pack_core_blobs(mm_dtype, w, id_feat, img_feat, txt_feat, core):
    """Merge weights + identity + per-core activations into two blobs so
    the kernel needs only two dma_starts (one per HWDGE engine)."""
    xin_a, xin_b = pack_core_inputs(mm_dtype, id_feat, img_feat, txt_feat,
                                    core)
    blk_a = np.concatenate([w["wcat_a"], w["ident"], xin_a], axis=1)
    blk_b = np.concatenate([w["wcat_b"], xin_b], axis=1)
    return np.ascontiguousarray(blk_a), np.ascontiguousarray(blk_b)


def pack_core_inputs(mm_dtype, id_feat, img_feat, txt_feat, core):
    """FT-layout per-core activations: xin_a [128, N], xin_b [65, N]."""
    sl = slice(core * BC, (core + 1) * BC)

    def ft(x):  # [BC, LK, D] -> [D, N] with n = l*BC + b
        return np.ascontiguousarray(
            x[sl, 0:LK, :].transpose(2, 1, 0).reshape(D, N))

    ddt = _np_dtype(mm_dtype)
    xin_a = np.concatenate([ft(id_feat), ft(img_feat)], axis=0).astype(ddt)
    xin_b = np.concatenate(
        [ft(txt_feat), np.ones((1, N), dtype=_f32)], axis=0).astype(ddt)
    return xin_a, xin_b


def numpy_forward(xin_a, xin_b, w):
    """Pure-numpy mirror of the device kernel (for validating the packing)."""
    xin = np.concatenate([xin_a, xin_b], axis=0).astype(_f32)   # [193, N]
    Wbig = np.concatenate(
        [w["wcat_a"][:, 0:1040], w["wcat_b"]], axis=0).astype(_f32)
    watt2 = w["wcat_a"][:, 1040:1168].astype(_f32)
    wap = np.concatenate([watt2[:, 0:64], watt2[:, 64:128]], axis=0)
    batt = w["wcat_a"][64, 1168:1232].astype(_f32)
    wgs2 = w["wcat_a"][0:64, 1232:2000].astype(_f32)

    P = xin.T @ Wbig                                      # [N, 1040]
    sel = P[:, 256:272]                                   # [N, 16]
    task = np.zeros((BC, D), dtype=_f32)
    att_all = np.zeros((2, 128, LK, D), dtype=_f32)
    comb_keep = np.zeros((2, 128, 256), dtype=_f32)
    for t in range(NT):
        rows = slice(t * 128, (t + 1) * 128)
        l, j = t // 2, t % 2
        E = P[rows, 0:256].reshape(128, D, 4)
        gs = (E * sel[rows, 0:4].reshape(128, 1, 4)).sum(-1)      # [128, 64]
        Tg = P[rows, 272:1040].reshape(128, 3, D, 4).copy()
        Tg += (gs @ wgs2).reshape(128, 3, D, 4)
        combined = np.zeros((128, 4, D), dtype=_f32)              # g-major
        for g in range(3):
            combined[:, g, :] = (
                Tg[:, g] * sel[rows, 4 * (g + 1):4 * (g + 2)].reshape(128, 1, 4)
            ).sum(-1)
        combined[:, 3, :] = gs
        cflat = combined.reshape(128, 256)
        logits = cflat @ wap + batt
        ex = np.exp(logits)
        att_all[j, :, l, :] = ex / ex.sum(-1, keepdims=True)
        if l == 0:
            comb_keep[j] = cflat
    for j in range(2):
        prod = comb_keep[j] * att_all[j].reshape(128, 256)
        task[j * 128:(j + 1) * 128] = prod.reshape(128, 4, D).sum(1)
    return task


# ---------------------------------------------------------------------------
# Bass program
# ---------------------------------------------------------------------------

def build_program(mm_dtype="bf16"):
    """Build the Bass/Tile program. Returns (nc, out_name)."""
    import concourse.bass as bass
    import concourse.bacc as bacc
    import concourse.mybir as mybir
    import concourse.tile as tile
    from contextlib import ExitStack

    f32 = mybir.dt.float32
    mmdt = {"f32": mybir.dt.float32,
            "f32r": mybir.dt.float32r,
            "bf16": mybir.dt.bfloat16}[mm_dtype]
    ddt = mybir.dt.bfloat16 if mm_dtype == "bf16" else f32
    cdt = mybir.dt.bfloat16 if mm_dtype == "bf16" else f32  # combine path

    nc = bacc.Bacc("TRN2", target_bir_lowering=False, debug=False)

    blk_a = nc.dram_tensor("blk_a", [KA, WCAT + 128 + N], ddt,
                           kind="ExternalInput").ap()
    blk_b = nc.dram_tensor("blk_b", [KB, WC + N], ddt,
                           kind="ExternalInput").ap()
    out = nc.dram_tensor("task", [BC, D], f32, kind="ExternalOutput").ap()

    Exp = mybir.ActivationFunctionType.Exp
    mult = mybir.AluOpType.mult
    add_op = mybir.AluOpType.add
    AX = mybir.AxisListType.X
    PSUM = bass.MemorySpace.PSUM

    with tile.TileContext(nc) as tc, ExitStack() as ctx:
        wp = ctx.enter_context(tc.tile_pool(name="w", bufs=1))
        work = ctx.enter_context(tc.tile_pool(name="work", bufs=3))
        keep = ctx.enter_context(tc.tile_pool(name="keep", bufs=1))
        pbe_pool = ctx.enter_context(tc.tile_pool(name="pbe", bufs=2, space=PSUM))
        pbt_pool = ctx.enter_context(tc.tile_pool(name="pbt", bufs=2, space=PSUM))
        ps_small = ctx.enter_context(tc.tile_pool(name="pssm", bufs=1, space=PSUM))

        # --- two batched input DMAs, one per HWDGE engine (serial
        # PSEUDO_DMA descriptor prep is ~0.7us per dma_start)
        ba = wp.tile([KA, WCAT + 128 + N], ddt, tag="ba")
        bb = wp.tile([KB, WC + N], ddt, tag="bb")
        # split blk_a so the weights + first two row-tiles' activations
        # land before the tail of xin: the first matmuls start ~2us earlier
        cut = WCAT + 128 + 256
        nc.sync.dma_start(ba[:, 0:cut], blk_a[:, 0:cut])
        nc.sync.dma_start(ba[:, cut:], blk_a[:, cut:])
        nc.scalar.dma_start(bb[:], blk_b[:])
        bwa = ba[:, 0:WCAT]
        idn = ba[:, WCAT:WCAT + 128]
        xa0 = ba[:, WCAT + 128:]
        bwb = bb[:, 0:WC]
        xb0 = bb[:, WC:]

        if mm_dtype == "f32r":
            # fp32r matmul operands must be produced by a rounding
            # instruction; DMA cannot round, so cast-copy once.
            wa = wp.tile([KA, WC], mmdt, tag="wa")
            wb = wp.tile([KB, WC], mmdt, tag="wb")
            wt = wp.tile([128, 128], mmdt, tag="wt")
            btr_t = wp.tile([65, 64], mmdt, tag="btr")
            wg = wp.tile([64, 768], mmdt, tag="wg")
            xa = wp.tile([KA, N], mmdt, tag="xa")
            xb = wp.tile([KB, N], mmdt, tag="xb")
            nc.scalar.copy(wa[:], bwa[:, 0:WC])
            nc.scalar.copy(wb[:], bwb)
            nc.vector.tensor_copy(wt[:], bwa[:, 1040:1168])
            nc.vector.tensor_copy(btr_t[64:65, :], bwa[64:65, 1168:1232])
            nc.vector.tensor_copy(wg[:], bwa[0:64, 1232:2000])
            btr = btr_t[64:65, :]
            nc.vector.tensor_copy(xa[:], xa0)
            nc.vector.tensor_copy(xb[:], xb0)
        else:
            wa = bwa[:, 0:WC]
            wb = bwb
            wt = bwa[:, 1040:1168]
            btr = bwa[64:65, 1168:1232]
            wg = bwa[0:64, 1232:2000]
            xa, xb = xa0, xb0

        # task accumulators (built incrementally as each slab's attention
        # finishes: task_j = sum_l comb_j[l-block] * att_(2l+j))
        task_acc = [keep.tile([128, D], f32, tag=f"task{j}", name=f"task{j}")
                    for j in range(2)]
        # per-tile persistent tiles: combined (gs block written in phase 1,
        # gate blocks in phase 2), gate logits, transposed gate_share
        comb = [keep.tile([128, 256], cdt, tag=f"comb{t}", name=f"comb{t}")
                for t in range(NT)]
        sels = [keep.tile([128, 16], f32, tag=f"sel{t}", name=f"sel{t}")
                for t in range(NT)]
        gsts = [keep.tile([64, 128], mmdt, tag=f"gst{t}", name=f"gst{t}")
                for t in range(NT)]

        # Phase 1: per tile, compute E + gate logits + the feature part of
        # the task experts (PE-dense, independent), and gate_share^T.
        # PSUM slot tags are shared across phases to stay within 8 banks:
        #   "ps1" (1 bank x2): pb_e (ph1) / ct_ps (ph2)
        #   "ps2" (1 bank x2): gst_ps (ph1) / att_ps (ph2)
        #   "pbt" (2 banks x2): pb_t (spans both phases)
        pb_ts = []
        for t in range(NT):
            cols = bass.ts(t, 128)
            pb_e = pbe_pool.tile([128, 272], f32, tag="ps1", name="pbe")
            pb_t = pbt_pool.tile([128, 768], f32, tag="pbt", name="pbt")
            pb_ts.append(pb_t)
            gst_ps = ps_small.tile([64, 128], cdt, tag="ps2", name="gstps")

            nc.tensor.matmul(pb_e[:, 0:272], xa[:, cols], wa[:, 0:272],
                             start=True, stop=False)
            nc.tensor.matmul(pb_e[:, 0:272], xb[:, cols], wb[:, 0:272],
                             start=False, stop=True)
            nc.tensor.matmul(pb_t[:, 0:512], xa[:, cols], wa[:, 272:784],
                             start=True, stop=False)
            nc.tensor.matmul(pb_t[:, 512:768], xa[:, cols], wa[:, 784:1040],
                             start=True, stop=False)
            nc.tensor.matmul(pb_t[:, 0:512], xb[:, cols], wb[:, 272:784],
                             start=False, stop=False)
            nc.tensor.matmul(pb_t[:, 512:768], xb[:, cols], wb[:, 784:1040],
                             start=False, stop=False)

            nc.scalar.copy(sels[t][:], pb_e[:, 256:272])

            # gate_share = sum_e E[:, d, e] * sel_s[:, e]  -> combined[192:256]
            prod_s = work.tile([128, D, 4], cdt, tag="prods")
            sel_s_b = sels[t][:, 0:4].unsqueeze(1).broadcast_to((128, D, 4))
            nc.vector.tensor_tensor(prod_s[:], pb_e[:, 0:256].rearrange(
                "p (d e) -> p d e", e=4), sel_s_b, op=mult)
            # e-reduction as pairwise adds on the (otherwise idle) GpSimd
            # engine: frees ~1.2us/tile of VectorE time vs reduce_sum.
            hs = work.tile([128, D, 2], cdt, tag="hs")
            nc.gpsimd.tensor_add(hs[:], prod_s[:, :, 0:2], prod_s[:, :, 2:4])
            nc.gpsimd.tensor_add(comb[t][:, 192:256], hs[:, :, 0], hs[:, :, 1])

            nc.tensor.matmul(gst_ps[:], comb[t][:, 192:256], idn[:],
                             is_transpose=True, start=True, stop=True)
            nc.scalar.copy(gsts[t][:], gst_ps[:])

        # Phase 2: add the gate_share contribution to the task experts,
        # gate-combine, attention, softmax.
        for t in range(NT):
            l, j = t // 2, t % 2
            cols = bass.ts(t, 128)
            pb_t = pb_ts[t]
            ct_ps = ps_small.tile([128, 256], cdt, tag="ps1", name="ctps")
            att_ps = ps_small.tile([128, 64], f32, tag="ps2", name="attps")

            nc.tensor.matmul(pb_t[:, 0:512], gsts[t][:], wg[:, 0:512],
                             start=False, stop=True)
            nc.tensor.matmul(pb_t[:, 512:768], gsts[t][:], wg[:, 512:768],
                             start=False, stop=True)

            # task gates: g_k = sum_e T_k[:, d, e] * sel_k[:, e]
            prod_t = work.tile([128, 3, D, 4], cdt, tag="prodt")
            sel_t_b = sels[t][:, 4:16].rearrange("p (g e) -> p g e", e=4) \
                .unsqueeze(2).broadcast_to((128, 3, D, 4))
            nc.vector.tensor_tensor(
                prod_t[:], pb_t[:, 0:768].rearrange(
                    "p (g d e) -> p g d e", g=3, e=4), sel_t_b, op=mult)
            ht = work.tile([128, 3, D, 2], cdt, tag="ht")
            nc.gpsimd.tensor_add(ht[:], prod_t[:, :, :, 0:2],
                                 prod_t[:, :, :, 2:4])
            nc.gpsimd.tensor_add(
                comb[t][:, 0:192].rearrange("p (g d) -> p g d", d=64),
                ht[:, :, :, 0], ht[:, :, :, 1])

            # attention: transpose combined, matmul Watt (+batt via ones row)
            nc.tensor.matmul(ct_ps[:, 0:128], comb[t][:, 0:128], idn[:],
                             is_transpose=True, start=True, stop=False)
            nc.tensor.matmul(ct_ps[:, 128:256], comb[t][:, 128:256], idn[:],
                             is_transpose=True, start=False, stop=True)
            ct = work.tile([128, 256], mmdt, tag="ct")
            nc.scalar.copy(ct[:, 0:128], ct_ps[:, 0:128])
            nc.scalar.copy(ct[:, 128:256], ct_ps[:, 128:256])
            nc.tensor.matmul(att_ps[:], ct[:, 0:128], wt[:, 0:64],
                             start=True, stop=False)
            nc.tensor.matmul(att_ps[:], ct[:, 128:256], wt[:, 64:128],
                             start=False, stop=False)
            nc.tensor.matmul(att_ps[:], xb[64:65, cols], btr,
                             start=False, stop=True)

            # softmax along D (no max subtraction: logits are tiny), fused
            # into the task accumulation: acc_j += (ex * rinv) * comb_j[l]
            ex = work.tile([128, D], f32, tag="ex")
            sumex = work.tile([128, 1], f32, tag="sumex")
            nc.scalar.activation(ex[:], att_ps[:], Exp, accum_out=sumex[:])
            rinv = work.tile([128, 1], f32, tag="rinv")
            nc.vector.reciprocal(rinv[:], sumex[:])
            term = task_acc[j] if l == 0 else work.tile(
                [128, D], f32, tag="term")
            nc.vector.scalar_tensor_tensor(
                term[:], ex[:], rinv[:], comb[j][:, l * D:(l + 1) * D],
                op0=mult, op1=mult)
            if l > 0:
                nc.vector.tensor_tensor(task_acc[j][:], task_acc[j][:],
                                        term[:], op=add_op)

        for j in range(2):
            nc.sync.dma_start(out[j * 128:(j + 1) * 128, :], task_acc[j][:])

    nc.compile()
    return nc, "task"


_PROGRAM_CACHE = {}


def _get_program(mm_dtype):
    if mm_dtype not in _PROGRAM_CACHE:
        _PROGRAM_CACHE[mm_dtype] = build_program(mm_dtype)
    return _PROGRAM_CACHE[mm_dtype]


def run_on_device(inputs, mm_dtype="bf16", trace=False):
    """Shard, run on 8 cores, return (task_full [B, D], BassKernelResults)."""
    from concourse.bass_utils import run_bass_kernel_spmd

    nc, out_name = _get_program(mm_dtype)
    w = pack_weights(mm_dtype, **{k: inputs[k] for k in (
        "Ws", "bs", "Wt1", "bt1", "Wt2", "bt2", "Wt3", "bt3",
        "Wgs", "bgs", "Wg1", "bg1", "Wg2", "bg2", "Wg3", "bg3",
        "Watt", "batt")})
    in_maps = []
    for c in range(NCORES):
        blk_a, blk_b = pack_core_blobs(
            mm_dtype, w, inputs["id_feat"], inputs["img_feat"],
            inputs["txt_feat"], c)
        in_maps.append({"blk_a": blk_a, "blk_b": blk_b})
    res = run_bass_kernel_spmd(nc, in_maps, core_ids=list(range(NCORES)),
                               trace=trace)
    task_full = np.concatenate(
        [res.results[c][out_name] for c in range(NCORES)], axis=0)
    return task_full, res


def kernel(**inputs):
    inputs = {k: np.asarray(v, dtype=np.float32) for k, v in inputs.items()}
    task_full, _ = run_on_device(inputs, mm_dtype="bf16")
    out = np.broadcast_to(task_full[:, None, :], (B, OUT_L, D))
    return np.ascontiguousarray(out)


# revision 29
# speedup vs baseline: 1.0131x; 1.0131x over previous
"""Trainium2 Bass kernel for nn_MultiScaleFusionLayer (moe_routing).

Mathematical structure of the reference:
  - Every stage (expert matmuls, gate logits, mixture combine, attention
    softmax) is position-wise along L.
  - The final output is `task` (computed from gates at l=0 and attention
    scores at l=0..3) broadcast along L=100.
  => Only input positions l in {0,1,2,3} ever influence the output.

Strategy:
  - Host slices inputs to l<4 and shards batch B=2048 across 8 cores
    (256 rows/core, 4 positions => N=1024 "rows" per core, l-major).
  - Activations are shipped feature-on-partition (FT layout) with a ones
    row appended, so every bias folds into the matmuls and matmul lhsT
    (stationary operand) never needs on-device transposition.
  - One fused weight matrix computes, per 128-row tile: shared experts E,
    all 16 gate logits, and the id/img/txt-feature parts of all task
    experts in two K-accumulated matmuls per PSUM bank. The gate_share
    contribution to the task experts is added with one PE transpose + two
    accumulating matmuls.
  - Expert mixture (per-row gating): DVE multiplies by broadcast gate
    columns; the 4-expert reduction runs as pairwise adds on the
    otherwise-idle GpSimd engine. Attention runs as 2 PE transposes +
    matmul (bias via a ones-row K=1 matmul) + softmax (no max
    subtraction: logits are tiny) fused into an incremental task
    accumulation, so nothing but the output DMA remains after the loop.
  - Device returns task [256, 64] per core; host broadcasts to L=100.

Layouts: expert blocks are d-major (col = d*4 + e) so the e-reduction is
an innermost-axis reduce; `combined`/attention use g-major blocks
(col = g*64 + d) so every reduce/copy output is contiguous.

mm_dtype modes: "f32" (exact), "f32r" (fp32 data, 1 cycle/row matmuls,
requires on-device rounding copies), "bf16" (host-cast inputs, fastest).
"""

import sys

if "/opt/trn_rl_repo" not in sys.path:
    sys.path.insert(0, "/opt/trn_rl_repo")

import numpy as np

B, L, D = 2048, 100, 64
OUT_L = 100
NCORES = 8
BC = B // NCORES          # 256 batch rows per core
LK = 4                    # positions that matter
N = BC * LK               # 1024 rows per core (n = l*BC + b)
NT = N // 128             # 8 row-tiles of 128 rows
KA, KB = 128, 65          # xin partition split (192 features + ones)
WC = 1040                 # fused cols: E 256 | sels 16 | T1f 256 | T2f 256 | T3f 256
WCAT = 2000               # wcat_a cols: wbig 1040 | watt 128 | batt 64 | wgs2 768

_f32 = np.float32


def _np_dtype(mm_dtype):
    if mm_dtype == "bf16":
        import ml_dtypes
        return np.dtype(ml_dtypes.bfloat16)
    return np.dtype(np.float32)


def pack_weights(mm_dtype, Ws, bs, Wt1, bt1, Wt2, bt2, Wt3, bt3,
                 Wgs, bgs, Wg1, bg1, Wg2, bg2, Wg3, bg3, Watt, batt):
    """Build the fused device weight arrays (replicated per core)."""
    def blk(Wt):  # [E,Din,D] -> [Din, D*4] with col = d*4+e
        return np.ascontiguousarray(
            np.transpose(Wt, (1, 2, 0)).reshape(Wt.shape[1], -1))

    def bblk(bt):  # [E,D] -> [D*4]
        return np.ascontiguousarray(bt.T.reshape(-1))

    Wbig = np.zeros((193, WC), dtype=_f32)
    # cols: [E 0:256][sels 256:272][T1f 272:528][T2f 528:784][T3f 784:1040]
    Wbig[0:192, 0:256] = blk(Ws)
    Wbig[192, 0:256] = bblk(bs)
    Wbig[0:192, 256:260] = Wgs
    Wbig[192, 256:260] = bgs
    Wbig[0:64, 260:264] = Wg1
    Wbig[192, 260:264] = bg1
    Wbig[64:128, 264:268] = Wg2
    Wbig[192, 264:268] = bg2
    Wbig[128:192, 268:272] = Wg3
    Wbig[192, 268:272] = bg3
    Wbig[0:64, 272:528] = blk(Wt1)
    Wbig[192, 272:528] = bblk(bt1)
    Wbig[64:128, 528:784] = blk(Wt2)
    Wbig[192, 528:784] = bblk(bt2)
    Wbig[128:192, 784:1040] = blk(Wt3)
    Wbig[192, 784:1040] = bblk(bt3)

    wgs2 = np.concatenate([blk(Wt1), blk(Wt2), blk(Wt3)], axis=1)  # [64, 768]
    watt2 = np.concatenate([Watt[0:128], Watt[128:256]], axis=1)   # [128, 128]

    wcat_a = np.zeros((KA, WCAT), dtype=_f32)
    wcat_a[:, 0:1040] = Wbig[0:128]
    wcat_a[:, 1040:1168] = watt2
    wcat_a[64, 1168:1232] = batt  # row 64: matches xin_b's ones-row partition
    wcat_a[0:64, 1232:2000] = wgs2
    wcat_b = np.ascontiguousarray(Wbig[128:193])

    ddt = _np_dtype(mm_dtype)
    return {
        "wcat_a": wcat_a.astype(ddt),
        "wcat_b": wcat_b.astype(ddt),
        "ident": np.eye(128, dtype=_f32).astype(ddt),
    }


def pack_core_blobs(mm_dtype, w, id_feat, img_feat, txt_feat, core):
    """Merge weights + identity + per-core activations into two blobs so
    the kernel needs only two dma_starts (one per HWDGE engine)."""
    xin_a, xin_b = pack_core_inputs(mm_dtype, id_feat, img_feat, txt_feat,
                                    core)
    blk_a = np.concatenate([w["wcat_a"], w["ident"], xin_a], axis=1)
    blk_b = np.concatenate([w["wcat_b"], xin_b], axis=1)
    return np.ascontiguousarray(blk_a), np.ascontiguousarray(blk_b)


def pack_core_inputs(mm_dtype, id_feat, img_feat, txt_feat, core):
    """FT-layout per-core activations: xin_a [128, N], xin_b [65, N]."""
    sl = slice(core * BC, (core + 1) * BC)

    def ft(x):  # [BC, LK, D] -> [D, N] with n = l*BC + b
        return np.ascontiguousarray(
            x[sl, 0:LK, :].transpose(2, 1, 0).reshape(D, N))

    ddt = _np_dtype(mm_dtype)
    xin_a = np.concatenate([ft(id_feat), ft(img_feat)], axis=0).astype(ddt)
    xin_b = np.concatenate(
        [ft(txt_feat), np.ones((1, N), dtype=_f32)], axis=0).astype(ddt)
    return xin_a, xin_b


def numpy_forward(xin_a, xin_b, w):
    """Pure-numpy mirror of the device kernel (for validating the packing)."""
    xin = np.concatenate([xin_a, xin_b], axis=0).astype(_f32)   # [193, N]
    Wbig = np.concatenate(
        [w["wcat_a"][:, 0:1040], w["wcat_b"]], axis=0).astype(_f32)
    watt2 = w["wcat_a"][:, 1040:1168].astype(_f32)
    wap = np.concatenate([watt2[:, 0:64], watt2[:, 64:128]], axis=0)
    batt = w["wcat_a"][64, 1168:1232].astype(_f32)
    wgs2 = w["wcat_a"][0:64, 1232:2000].astype(_f32)

    P = xin.T @ Wbig                                      # [N, 1040]
    sel = P[:, 256:272]                                   # [N, 16]
    task = np.zeros((BC, D), dtype=_f32)
    att_all = np.zeros((2, 128, LK, D), dtype=_f32)
    comb_keep = np.zeros((2, 128, 256), dtype=_f32)
    for t in range(NT):
        rows = slice(t * 128, (t + 1) * 128)
        l, j = t // 2, t % 2
        E = P[rows, 0:256].reshape(128, D, 4)
        gs = (E * sel[rows, 0:4].reshape(128, 1, 4)).sum(-1)      # [128, 64]
        Tg = P[rows, 272:1040].reshape(128, 3, D, 4).copy()
        Tg += (gs @ wgs2).reshape(128, 3, D, 4)
        combined = np.zeros((128, 4, D), dtype=_f32)              # g-major
        for g in range(3):
            combined[:, g, :] = (
                Tg[:, g] * sel[rows, 4 * (g + 1):4 * (g + 2)].reshape(128, 1, 4)
            ).sum(-1)
        combined[:, 3, :] = gs
        cflat = combined.reshape(128, 256)
        logits = cflat @ wap + batt
        ex = np.exp(logits)
        att_all[j, :, l, :] = ex / ex.sum(-1, keepdims=True)
        if l == 0:
            comb_keep[j] = cflat
    for j in range(2):
        prod = comb_keep[j] * att_all[j].reshape(128, 256)
        task[j * 128:(j + 1) * 128] = prod.reshape(128, 4, D).sum(1)
    return task


# ---------------------------------------------------------------------------
# Bass program
# ---------------------------------------------------------------------------

def build_program(mm_dtype="bf16"):
    """Build the Bass/Tile program. Returns (nc, out_name)."""
    import concourse.bass as bass
    import concourse.bacc as bacc
    import concourse.mybir as mybir
    import concourse.tile as tile
    from contextlib import ExitStack

    f32 = mybir.dt.float32
    mmdt = {"f32": mybir.dt.float32,
            "f32r": mybir.dt.float32r,
            "bf16": mybir.dt.bfloat16}[mm_dtype]
    ddt = mybir.dt.bfloat16 if mm_dtype == "bf16" else f32
    cdt = mybir.dt.bfloat16 if mm_dtype == "bf16" else f32  # combine path

    nc = bacc.Bacc("TRN2", target_bir_lowering=False, debug=False)

    blk_a = nc.dram_tensor("blk_a", [KA, WCAT + 128 + N], ddt,
                           kind="ExternalInput").ap()
    blk_b = nc.dram_tensor("blk_b", [KB, WC + N], ddt,
                           kind="ExternalInput").ap()
    out = nc.dram_tensor("task", [BC, D], f32, kind="ExternalOutput").ap()

    Exp = mybir.ActivationFunctionType.Exp
    mult = mybir.AluOpType.mult
    add_op = mybir.AluOpType.add
    AX = mybir.AxisListType.X
    PSUM = bass.MemorySpace.PSUM

    with tile.TileContext(nc) as tc, ExitStack() as ctx:
        wp = ctx.enter_context(tc.tile_pool(name="w", bufs=1))
        work = ctx.enter_context(tc.tile_pool(name="work", bufs=3))
        keep = ctx.enter_context(tc.tile_pool(name="keep", bufs=1))
        pbe_pool = ctx.enter_context(tc.tile_pool(name="pbe", bufs=2, space=PSUM))
        pbt_pool = ctx.enter_context(tc.tile_pool(name="pbt", bufs=2, space=PSUM))
        ps_small = ctx.enter_context(tc.tile_pool(name="pssm", bufs=1, space=PSUM))

        # --- two batched input DMAs, one per HWDGE engine (serial
        # PSEUDO_DMA descriptor prep is ~0.7us per dma_start)
        ba = wp.tile([KA, WCAT + 128 + N], ddt, tag="ba")
        bb = wp.tile([KB, WC + N], ddt, tag="bb")
        # split blk_a so the weights + first two row-tiles' activations
        # land before the tail of xin: the first matmuls start ~2us earlier
        cut = WCAT + 128 + 256
        nc.sync.dma_start(ba[:, 0:cut], blk_a[:, 0:cut])
        nc.sync.dma_start(ba[:, cut:], blk_a[:, cut:])
        nc.scalar.dma_start(bb[:], blk_b[:])
        bwa = ba[:, 0:WCAT]
        idn = ba[:, WCAT:WCAT + 128]
        xa0 = ba[:, WCAT + 128:]
        bwb = bb[:, 0:WC]
        xb0 = bb[:, WC:]

        if mm_dtype == "f32r":
            # fp32r matmul operands must be produced by a rounding
            # instruction; DMA cannot round, so cast-copy once.
            wa = wp.tile([KA, WC], mmdt, tag="wa")
            wb = wp.tile([KB, WC], mmdt, tag="wb")
            wt = wp.tile([128, 128], mmdt, tag="wt")
            btr_t = wp.tile([65, 64], mmdt, tag="btr")
            wg = wp.tile([64, 768], mmdt, tag="wg")
            xa = wp.tile([KA, N], mmdt, tag="xa")
            xb = wp.tile([KB, N], mmdt, tag="xb")
            nc.scalar.copy(wa[:], bwa[:, 0:WC])
            nc.scalar.copy(wb[:], bwb)
            nc.vector.tensor_copy(wt[:], bwa[:, 1040:1168])
            nc.vector.tensor_copy(btr_t[64:65, :], bwa[64:65, 1168:1232])
            nc.vector.tensor_copy(wg[:], bwa[0:64, 1232:2000])
            btr = btr_t[64:65, :]
            nc.vector.tensor_copy(xa[:], xa0)
            nc.vector.tensor_copy(xb[:], xb0)
        else:
            wa = bwa[:, 0:WC]
            wb = bwb
            wt = bwa[:, 1040:1168]
            btr = bwa[64:65, 1168:1232]
            wg = bwa[0:64, 1232:2000]
            xa, xb = xa0, xb0

        # task accumulators (built incrementally as each slab's attention
        # finishes: task_j = sum_l comb_j[l-block] * att_(2l+j))
        task_acc = [keep.tile([128, D], f32, tag=f"task{j}", name=f"task{j}")
                    for j in range(2)]
        # per-tile persistent tiles: combined (gs block written in phase 1,
        # gate blocks in phase 2), gate logits, transposed gate_share
        comb = [keep.tile([128, 256], cdt, tag=f"comb{t}", name=f"comb{t}")
                for t in range(NT)]
        sels = [keep.tile([128, 16], f32, tag=f"sel{t}", name=f"sel{t}")
                for t in range(NT)]
        gsts = [keep.tile([64, 128], mmdt, tag=f"gst{t}", name=f"gst{t}")
                for t in range(NT)]

        # Phase 1: per tile, compute E + gate logits + the feature part of
        # the task experts (PE-dense, independent), and gate_share^T.
        # PSUM slot tags are shared across phases to stay within 8 banks:
        #   "ps1" (1 bank x2): pb_e (ph1) / ct_ps (ph2)
        #   "ps2" (1 bank x2): gst_ps (ph1) / att_ps (ph2)
        #   "pbt" (2 banks x2): pb_t (spans both phases)
        pb_ts = []
        for t in range(NT):
            cols = bass.ts(t, 128)
            pb_e = pbe_pool.tile([128, 272], f32, tag="ps1", name="pbe")
            pb_t = pbt_pool.tile([128, 768], f32, tag="pbt", name="pbt")
            pb_ts.append(pb_t)
            gst_ps = ps_small.tile([64, 128], cdt, tag="ps2", name="gstps")

            nc.tensor.matmul(pb_e[:, 0:272], xa[:, cols], wa[:, 0:272],
                             start=True, stop=False)
            nc.tensor.matmul(pb_e[:, 0:272], xb[:, cols], wb[:, 0:272],
                             start=False, stop=True)
            nc.tensor.matmul(pb_t[:, 0:512], xa[:, cols], wa[:, 272:784],
                             start=True, stop=False)
            nc.tensor.matmul(pb_t[:, 512:768], xa[:, cols], wa[:, 784:1040],
                             start=True, stop=False)
            nc.tensor.matmul(pb_t[:, 0:512], xb[:, cols], wb[:, 272:784],
                             start=False, stop=False)
            nc.tensor.matmul(pb_t[:, 512:768], xb[:, cols], wb[:, 784:1040],
                             start=False, stop=False)

            nc.scalar.copy(sels[t][:], pb_e[:, 256:272])

            # gate_share = sum_e E[:, d, e] * sel_s[:, e]  -> combined[192:256]
            prod_s = work.tile([128, D, 4], cdt, tag="prods")
            sel_s_b = sels[t][:, 0:4].unsqueeze(1).broadcast_to((128, D, 4))
            nc.vector.tensor_tensor(prod_s[:], pb_e[:, 0:256].rearrange(
                "p (d e) -> p d e", e=4), sel_s_b, op=mult)
            # e-reduction as pairwise adds on the (otherwise idle) GpSimd
            # engine: frees ~1.2us/tile of VectorE time vs reduce_sum.
            hs = work.tile([128, D, 2], cdt, tag="hs")
            nc.gpsimd.tensor_add(hs[:], prod_s[:, :, 0:2], prod_s[:, :, 2:4])
            nc.gpsimd.tensor_add(comb[t][:, 192:256], hs[:, :, 0], hs[:, :, 1])

            nc.tensor.matmul(gst_ps[:], comb[t][:, 192:256], idn[:],
                             is_transpose=True, start=True, stop=True)
            nc.scalar.copy(gsts[t][:], gst_ps[:])

        # Phase 2: add the gate_share contribution to the task experts,
        # gate-combine, attention, softmax.
        for t in range(NT):
            l, j = t // 2, t % 2
            cols = bass.ts(t, 128)
            pb_t = pb_ts[t]
            ct_ps = ps_small.tile([128, 256], cdt, tag="ps1", name="ctps")
            att_ps = ps_small.tile([128, 64], f32, tag="ps2", name="attps")

            nc.tensor.matmul(pb_t[:, 0:512], gsts[t][:], wg[:, 0:512],
                             start=False, stop=True)
            nc.tensor.matmul(pb_t[:, 512:768], gsts[t][:], wg[:, 512:768],
                             start=False, stop=True)

            # task gates: g_k = sum_e T_k[:, d, e] * sel_k[:, e]
            prod_t = work.tile([128, 3, D, 4], cdt, tag="prodt")
            sel_t_b = sels[t][:, 4:16].rearrange("p (g e) -> p g e", e=4) \
                .unsqueeze(2).broadcast_to((128, 3, D, 4))
            nc.vector.tensor_tensor(
                prod_t[:], pb_t[:, 0:768].rearrange(
                    "p (g d e) -> p g d e", g=3, e=4), sel_t_b, op=mult)
            ht = work.tile([128, 3, D, 2], cdt, tag="ht")
            nc.gpsimd.tensor_add(ht[:], prod_t[:, :, :, 0:2],
                                 prod_t[:, :, :, 2:4])
            nc.gpsimd.tensor_add(
                comb[t][:, 0:192].rearrange("p (g d) -> p g d", d=64),
                ht[:, :, :, 0], ht[:, :, :, 1])

            # attention: transpose combined, matmul Watt (+batt via ones row)
            nc.tensor.matmul(ct_ps[:, 0:128], comb[t][:, 0:128], idn[:],
                             is_transpose=True, start=True, stop=False)
            nc.tensor.matmul(ct_ps[:, 128:256], comb[t][:, 128:256], idn[:],
                             is_transpose=True, start=False, stop=True)
            ct = work.tile([128, 256], mmdt, tag="ct")
            nc.scalar.copy(ct[:, 0:128], ct_ps[:, 0:128])
            nc.scalar.copy(ct[:, 128:256], ct_ps[:, 128:256])
            nc.tensor.matmul(att_ps[:], ct[:, 0:128], wt[:, 0:64],
                             start=True, stop=False)
            nc.tensor.matmul(att_ps[:], ct[:, 128:256], wt[:, 64:128],
                             start=False, stop=False)
            nc.tensor.matmul(att_ps[:], xb[64:65, cols], btr,
                             start=False, stop=True)

            # softmax along D (no max subtraction: logits are tiny), fused
            # into the task accumulation: acc_j += (ex * rinv) * comb_j[l]
            ex = work.tile([128, D], f32, tag="ex")
            sumex = work.tile([128, 1], f32, tag="sumex")
            nc.scalar.activation(ex[:], att_ps[:], Exp, accum_out=sumex[:])
            rinv = work.tile([128, 1], f32, tag="rinv")
            nc.vector.reciprocal(rinv[:], sumex[:])
            term = task_acc[j] if l == 0 else work.tile(
                [128, D], f32, tag="term")
            nc.vector.scalar_tensor_tensor(
                term[:], ex[:], rinv[:], comb[j][:, l * D:(l + 1) * D],
                op0=mult, op1=mult)
            if l > 0:
                nc.vector.tensor_tensor(task_acc[j][:], task_acc[j][:],
                                        term[:], op=add_op)

        for j in range(2):
            nc.sync.dma_start(out[j * 128:(j + 1) * 128, :], task_acc[j][:])

    nc.compile()
    return nc, "task"


_PROGRAM_CACHE = {}


def _get_program(mm_dtype):
    if mm_dtype not in _PROGRAM_CACHE:
        _PROGRAM_CACHE[mm_dtype] = build_program(mm_dtype)
    return _PROGRAM_CACHE[mm_dtype]


def run_on_device(inputs, mm_dtype="bf16", trace=False):
    """Shard, run on 8 cores, return (task_full [B, D], BassKernelResults)."""
    from concourse.bass_utils import run_bass_kernel_spmd

    nc, out_name = _get_program(mm_dtype)
    w = pack_weights(mm_dtype, **{k: inputs[k] for k in (
        "Ws", "bs", "Wt1", "bt1", "Wt2", "bt2", "Wt3", "bt3",
        "Wgs", "bgs", "Wg1", "bg1", "Wg2", "bg2", "Wg3", "bg3",
        "Watt", "batt")})
    in_maps = []
    for c in range(NCORES):
        blk_a, blk_b = pack_core_blobs(
            mm_dtype, w, inputs["id_feat"], inputs["img_feat"],
            inputs["txt_feat"], c)
        in_maps.append({"blk_a": blk_a, "blk_b": blk_b})
    res = run_bass_kernel_spmd(nc, in_maps, core_ids=list(range(NCORES)),
                               trace=trace)
    task_full = np.concatenate(
        [res.results[c][out_name] for c in range(NCORES)], axis=0)
    return task_full, res


def kernel(**inputs):
    inputs = {k: np.asarray(v, dtype=np.float32) for k, v in inputs.items()}
    task_full, _ = run_on_device(inputs, mm_dtype="bf16")
    out = np.broadcast_to(task_full[:, None, :], (B, OUT_L, D))
    return np.ascontiguousarray(out)
